# revision 15
# baseline (speedup 1.0000x reference)
"""Trainium2 Bass kernel for nn_Attention_26079041421696.

Head-per-core tensor parallel (8 heads -> 8 NeuronCores). Per core
(n=4096 tokens, C=256 channels, dh=64):

    q = x @ wq, k = x @ wk, v = x @ wv       (1x1 conv slices)
    simT[j, i] = k_j . q_i                   (j-chunk on partitions)
    p = exp(SCALE * simT)
    oT[d, i]  = sum_j v[j, d] p[j, i]        (unnormalized; ones-column in
                                              v makes row 64 the softmax
                                              denominator for free)
Host epilogue: out = sum_h wo_h^T (oT_h / den_h) + b_out  (output
projection on the host -- O(n*C*dh) BLAS, saves PE slots + 1.5MB of
output DMA per core).

Schedule notes (tuned against neuron-profile traces):
  - j-outer / i-tile-pair loop: each j-chunk pair's kk stationaries serve
    2 i-tiles (4 sim matmuls) before reload -> LDWEIGHTS mostly hidden.
  - exp tiles split ScalarE (exact, fp16) vs VectorE 1-op Schraudolph
    fast-exp: p = bitcast_fp16(round_i16(A*sim + B)); the ~1.5% sawtooth
    is unbiased after softmax normalization.
  - acc drains deferred into the next group's jp==1 slot so they never
    block that group's exps in the in-order ScalarE stream.
"""

import numpy as np
import ml_dtypes

HEADS = 8
DH = 64
N_TOK = 4096
C_IN = 256
SCALE = DH ** -0.5
N_CORES = 8

LOG2E = 1.4426950408889634
A16 = 1024.0 * LOG2E * SCALE      # Schraudolph scale (applied to raw sim)
B16 = 15.0 * 1024.0 - 35.0        # exponent bias + fitted offset

_CACHE = {}


def build_nc():
    import concourse.bacc as bacc
    import concourse.mybir as mybir
    from concourse import tile

    bf16 = mybir.dt.bfloat16
    f16 = mybir.dt.float16
    f32 = mybir.dt.float32
    i16 = mybir.dt.int16
    Exp = mybir.ActivationFunctionType.Exp
    mult = mybir.AluOpType.mult
    addop = mybir.AluOpType.add

    nc = bacc.Bacc("TRN2", target_bir_lowering=False, debug=False)

    xT_d = nc.dram_tensor("xT", [C_IN, N_TOK], bf16, kind="ExternalInput")
    wqkv_d = nc.dram_tensor("wqkv", [128, 384], bf16, kind="ExternalInput")
    oT_d = nc.dram_tensor("oT", [DH + 1, N_TOK], f16, kind="ExternalOutput")

    with tile.TileContext(nc) as tc:
        with (
            tc.tile_pool(name="cpool", bufs=1) as cpool,
            tc.tile_pool(name="spool", bufs=2) as spool,
            tc.tile_pool(name="pspool", bufs=2, space="PSUM") as pspool,
        ):
            # ---- persistent SBUF tiles -------------------------------
            x0 = cpool.tile([128, N_TOK], bf16, tag="x0")
            x1 = cpool.tile([128, N_TOK], bf16, tag="x1")
            wqkv = cpool.tile([128, 384], bf16, tag="wqkv")
            qqT = cpool.tile([128, N_TOK], bf16, tag="qq")
            kkT = cpool.tile([128, N_TOK], bf16, tag="kk")
            v_sb = cpool.tile([128, 32 * 65], f16, tag="v")

            # preload the exp table set before any real work
            warm = cpool.tile([1, 8], f32, tag="warm")
            nc.vector.memset(warm[:], 0.0)
            nc.scalar.activation(warm[:], warm[:], Exp)

            nc.sync.dma_start(wqkv[:], wqkv_d[:])
            for ci in range(8):
                cs = slice(ci * 512, (ci + 1) * 512)
                nc.sync.dma_start(x0[:, cs], xT_d[0:128, cs])
                nc.gpsimd.dma_start(x1[:, cs], xT_d[128:256, cs])
            # ones-columns (col 64 of each 65-wide chunk) for the denominator
            ones_view = v_sb[:].rearrange("p (c d) -> p c d", d=65)[:, :, 64:65]
            nc.vector.memset(ones_view, 1.0)

            # ---- q/k production: col-tiled (q -> partitions 0:64, k -> 64:128)
            def emit_qk(t):
                sl = slice(t * 512, (t + 1) * 512)
                ps = pspool.tile([128, 512], f32, tag="acc", name=f"psqk{t}")
                nc.tensor.matmul(ps[0:64, :], wqkv[:, 0:64], x0[:, sl],
                                 start=True, stop=False, tile_position=(0, 0))
                nc.tensor.matmul(ps[64:128, :], wqkv[:, 128:192], x0[:, sl],
                                 start=True, stop=False, tile_position=(0, 64))
                nc.tensor.matmul(ps[0:64, :], wqkv[:, 64:128], x1[:, sl],
                                 start=False, stop=True, tile_position=(0, 0))
                nc.tensor.matmul(ps[64:128, :], wqkv[:, 192:256], x1[:, sl],
                                 start=False, stop=True, tile_position=(0, 64))
                nc.scalar.copy(qqT[0:64, sl], ps[0:64, :])
                nc.sync.dma_start(qqT[64:128, sl], qqT[0:64, sl])
                nc.vector.tensor_copy(kkT[64:128, sl], ps[64:128, :])
                nc.sync.dma_start(kkT[0:64, sl], kkT[64:128, sl])

            # ---- v production: blocks of 1024 tokens -----------------
            def emit_v(blk):
                psv = pspool.tile([128, 512], f32, tag="acc", name=f"psv{blk}")
                for c in range(8):
                    tck = blk * 8 + c
                    slt = slice(tck * 128, (tck + 1) * 128)
                    nc.tensor.matmul(psv[:, c * DH:(c + 1) * DH],
                                     x0[:, slt], wqkv[:, 256:320],
                                     start=True, stop=False)
                    nc.tensor.matmul(psv[:, c * DH:(c + 1) * DH],
                                     x1[:, slt], wqkv[:, 320:384],
                                     start=False, stop=True)
                vdst = v_sb[:, blk * 520:(blk + 1) * 520]
                vdst = vdst.rearrange("p (a b) -> p a b", b=65)[:, :, 0:DH]
                nc.vector.tensor_copy(
                    vdst, psv[:].rearrange("p (a b) -> p a b", b=DH))

            for t in range(8):
                emit_qk(t)
            for blk in range(4):
                emit_v(blk)

            # ---- attention: 4 groups x (2 i-tiles) x 16 j-chunk-pairs
            pending_ep = None
            for g in range(4):
                sA = slice((2 * g) * 512, (2 * g + 1) * 512)
                sB = slice((2 * g + 1) * 512, (2 * g + 2) * 512)
                accs = [
                    pspool.tile([DH + 1, 512], f32, tag="acc", name=f"acc{g}a"),
                    pspool.tile([DH + 1, 512], f32, tag="acc", name=f"acc{g}b"),
                ]
                started = [False, False]
                avq = []  # (acc_idx, v_ap, p_ap)

                def drain(keep, accs=accs, started=started, avq=avq,
                          final=False):
                    while len(avq) > keep:
                        ai, vs, ps_ = avq.pop(0)
                        is_last = final and not any(q[0] == ai for q in avq)
                        nc.tensor.matmul(accs[ai][:], vs, ps_,
                                         start=(not started[ai]),
                                         stop=is_last)
                        started[ai] = True

                for jp in range(16):
                    c0, c1 = 2 * jp, 2 * jp + 1
                    j0 = slice(c0 * 128, (c0 + 1) * 128)
                    j1 = slice(c1 * 128, (c1 + 1) * 128)
                    if jp == 1 and pending_ep is not None:
                        pending_ep()
                        pending_ep = None
                    # AVs from ~2 jps ago first: they fill the PE while this
                    # jp's sim tiles wait on exp completions
                    if jp % 2 == 1 or jp == 14:
                        drain(4 if jp == 14 else 8)
                    # sims: kk chunk stationaries each serve both i-tiles
                    pst = [
                        pspool.tile([128, 1024], f32, tag="sim", bufs=3,
                                    name=f"pst{g}_{jp}_{ti}")
                        for ti in (0, 1)
                    ]
                    for ti, si in ((0, sA), (1, sB)):
                        nc.tensor.matmul(pst[ti][:, 0:512], kkT[0:64, j0],
                                         qqT[0:64, si], start=True, stop=True)
                    for ti, si in ((0, sA), (1, sB)):
                        nc.tensor.matmul(pst[ti][:, 512:1024], kkT[64:128, j1],
                                         qqT[64:128, si], start=True, stop=True)
                    # exps
                    for ti in (0, 1):
                        t_glob = g * 32 + jp * 2 + ti
                        m = t_glob % 8
                        p_sb = spool.tile([128, 1024], f16, tag="p",
                                          bufs=10, name=f"p{g}_{jp}_{ti}")
                        if (t_glob % 2 == 0) or (t_glob % 16 == 7):
                            nc.scalar.activation(p_sb[:], pst[ti][:], Exp,
                                                 scale=SCALE)
                        else:
                            nc.vector.tensor_scalar(
                                p_sb[:].bitcast(i16), pst[ti][:],
                                A16, B16, mult, addop)
                        avq.append((ti, v_sb[:, c0 * 65:c0 * 65 + 65],
                                    p_sb[:, 0:512]))
                        avq.append((ti, v_sb[:, c1 * 65:c1 * 65 + 65],
                                    p_sb[:, 512:1024]))
                drain(0, final=True)

                def make_ep(g, accs):
                    def ep():
                        for ti in (0, 1):
                            osb = spool.tile([DH + 1, 512], f16, tag="osb",
                                             bufs=4, name=f"osb{g}_{ti}")
                            if ti == 0:
                                nc.scalar.copy(osb[:], accs[ti][:])
                            else:
                                nc.vector.tensor_copy(osb[:], accs[ti][:])
                            it = 2 * g + ti
                            nc.sync.dma_start(
                                oT_d[:, it * 512:(it + 1) * 512], osb[:])
                    return ep

                pending_ep = make_ep(g, accs)
            pending_ep()

    nc.compile()
    return nc


def make_in_maps(x, w_qkv):
    bf = ml_dtypes.bfloat16
    xf = np.asarray(x, np.float32).reshape(N_TOK, C_IN)
    xT = np.ascontiguousarray(xf.T).astype(bf)
    w_qkv = np.asarray(w_qkv, np.float32)
    in_maps = []
    for h in range(HEADS):
        wq = w_qkv[:, h * DH:(h + 1) * DH]
        wk = w_qkv[:, 512 + h * DH:512 + (h + 1) * DH]
        wv = w_qkv[:, 1024 + h * DH:1024 + (h + 1) * DH]
        wqkv_np = np.concatenate(
            [wq[:128], wq[128:], wk[:128], wk[128:], wv[:128], wv[128:]],
            axis=1).astype(bf)
        in_maps.append({"xT": xT, "wqkv": wqkv_np})
    return in_maps


def postprocess(results, w_out, b_out):
    """Host epilogue: normalize, project, sum heads, add bias."""
    w_out = np.asarray(w_out, np.float32)
    acc = np.zeros((N_TOK, C_IN), np.float32)
    for h in range(HEADS):
        o = np.asarray(results[h]["oT"], dtype=np.float32)
        oT = o[0:DH]                      # [64, n] unnormalized
        den = o[DH]                       # [n]
        acc += (oT / den[None, :]).T @ w_out[h * DH:(h + 1) * DH, :]
    out = acc + np.asarray(b_out, np.float32)[None, :]
    return out.astype(np.float32).reshape(1, 8, 16, 32, C_IN)


def kernel(x, w_qkv, w_out, b_out):
    from concourse.bass_utils import run_bass_kernel_spmd

    nc = _CACHE.get("nc")
    if nc is None:
        nc = build_nc()
        _CACHE["nc"] = nc
    in_maps = make_in_maps(x, w_qkv)
    res = run_bass_kernel_spmd(nc, in_maps, core_ids=list(range(N_CORES)))
    return postprocess(res.results, w_out, b_out)


# revision 16
# speedup vs baseline: 1.2017x; 1.2017x over previous
"""Trainium2 Bass kernel for nn_Attention_26079041421696.

Head-per-core tensor parallel (8 heads -> 8 NeuronCores). Per core
(n=4096 tokens, C=256 channels, dh=64):

    q = x @ wq, k = x @ wk, v = x @ wv       (1x1 conv slices)
    simT[j, i] = k_j . q_i                   (j-chunk on partitions)
    p = exp(SCALE * simT)
    oT[d, i]  = sum_j v[j, d] p[j, i]        (unnormalized; ones-column in
                                              v makes row 64 the softmax
                                              denominator for free)
Host epilogue: out = sum_h wo_h^T (oT_h / den_h) + b_out  (output
projection on the host -- O(n*C*dh) BLAS, saves PE slots + 1.5MB of
output DMA per core).

Schedule notes (tuned against neuron-profile traces):
  - j-outer / i-tile-pair loop: each j-chunk pair's kk stationaries serve
    2 i-tiles (4 sim matmuls) before reload -> LDWEIGHTS mostly hidden.
  - exp tiles split ScalarE (exact, fp16) vs VectorE 1-op Schraudolph
    fast-exp: p = bitcast_fp16(round_i16(A*sim + B)); the ~1.5% sawtooth
    is unbiased after softmax normalization.
  - acc drains deferred into the next group's jp==1 slot so they never
    block that group's exps in the in-order ScalarE stream.
"""

import numpy as np
import ml_dtypes

HEADS = 8
DH = 64
N_TOK = 4096
C_IN = 256
SCALE = DH ** -0.5
N_CORES = 8

LOG2E = 1.4426950408889634
A16 = 1024.0 * LOG2E * SCALE      # Schraudolph scale (applied to raw sim)
B16 = 15.0 * 1024.0 - 35.0        # exponent bias + fitted offset

_CACHE = {}


def build_nc():
    import concourse.bacc as bacc
    import concourse.mybir as mybir
    from concourse import tile

    bf16 = mybir.dt.bfloat16
    f16 = mybir.dt.float16
    f32 = mybir.dt.float32
    i16 = mybir.dt.int16
    Exp = mybir.ActivationFunctionType.Exp
    mult = mybir.AluOpType.mult
    addop = mybir.AluOpType.add

    nc = bacc.Bacc("TRN2", target_bir_lowering=False, debug=False)

    xT_d = nc.dram_tensor("xT", [C_IN, N_TOK], bf16, kind="ExternalInput")
    wqkv_d = nc.dram_tensor("wqkv", [128, 384], bf16, kind="ExternalInput")
    oT_d = nc.dram_tensor("oT", [DH + 1, N_TOK], f16, kind="ExternalOutput")

    with tile.TileContext(nc) as tc:
        with (
            tc.tile_pool(name="cpool", bufs=1) as cpool,
            tc.tile_pool(name="spool", bufs=2) as spool,
            tc.tile_pool(name="pspool", bufs=2, space="PSUM") as pspool,
        ):
            # ---- persistent SBUF tiles -------------------------------
            x0 = cpool.tile([128, N_TOK], bf16, tag="x0")
            x1 = cpool.tile([128, N_TOK], bf16, tag="x1")
            wqkv = cpool.tile([128, 384], bf16, tag="wqkv")
            qqT = cpool.tile([128, N_TOK], bf16, tag="qq")
            kkT = cpool.tile([128, N_TOK], bf16, tag="kk")
            v_sb = cpool.tile([128, 32 * 65], f16, tag="v")

            # preload the exp table set before any real work
            warm = cpool.tile([1, 8], f32, tag="warm")
            nc.vector.memset(warm[:], 0.0)
            nc.scalar.activation(warm[:], warm[:], Exp)

            nc.sync.dma_start(wqkv[:], wqkv_d[:])
            for ci in range(8):
                cs = slice(ci * 512, (ci + 1) * 512)
                nc.sync.dma_start(x0[:, cs], xT_d[0:128, cs])
                nc.sync.dma_start(x1[:, cs], xT_d[128:256, cs])
            # ones-columns (col 64 of each 65-wide chunk) for the denominator
            ones_view = v_sb[:].rearrange("p (c d) -> p c d", d=65)[:, :, 64:65]
            nc.vector.memset(ones_view, 1.0)

            # ---- q/k production: col-tiled (q -> partitions 0:64, k -> 64:128)
            def emit_qk(t):
                sl = slice(t * 512, (t + 1) * 512)
                ps = pspool.tile([128, 512], f32, tag="acc", name=f"psqk{t}")
                nc.tensor.matmul(ps[0:64, :], wqkv[:, 0:64], x0[:, sl],
                                 start=True, stop=False, tile_position=(0, 0))
                nc.tensor.matmul(ps[64:128, :], wqkv[:, 128:192], x0[:, sl],
                                 start=True, stop=False, tile_position=(0, 64))
                nc.tensor.matmul(ps[0:64, :], wqkv[:, 64:128], x1[:, sl],
                                 start=False, stop=True, tile_position=(0, 0))
                nc.tensor.matmul(ps[64:128, :], wqkv[:, 192:256], x1[:, sl],
                                 start=False, stop=True, tile_position=(0, 64))
                nc.scalar.copy(qqT[0:64, sl], ps[0:64, :])
                nc.sync.dma_start(qqT[64:128, sl], qqT[0:64, sl])
                nc.vector.tensor_copy(kkT[64:128, sl], ps[64:128, :])
                nc.sync.dma_start(kkT[0:64, sl], kkT[64:128, sl])

            # ---- v production: blocks of 1024 tokens -----------------
            def emit_v(blk):
                psv = pspool.tile([128, 512], f32, tag="acc", name=f"psv{blk}")
                for c in range(8):
                    tck = blk * 8 + c
                    slt = slice(tck * 128, (tck + 1) * 128)
                    nc.tensor.matmul(psv[:, c * DH:(c + 1) * DH],
                                     x0[:, slt], wqkv[:, 256:320],
                                     start=True, stop=False)
                    nc.tensor.matmul(psv[:, c * DH:(c + 1) * DH],
                                     x1[:, slt], wqkv[:, 320:384],
                                     start=False, stop=True)
                vdst = v_sb[:, blk * 520:(blk + 1) * 520]
                vdst = vdst.rearrange("p (a b) -> p a b", b=65)[:, :, 0:DH]
                nc.vector.tensor_copy(
                    vdst, psv[:].rearrange("p (a b) -> p a b", b=DH))

            for t in range(8):
                emit_qk(t)
            for blk in range(4):
                emit_v(blk)

            # ---- attention: 4 groups x (2 i-tiles) x 16 j-chunk-pairs
            pending_ep = None
            for g in range(4):
                sA = slice((2 * g) * 512, (2 * g + 1) * 512)
                sB = slice((2 * g + 1) * 512, (2 * g + 2) * 512)
                accs = [
                    pspool.tile([DH + 1, 512], f32, tag="acc", name=f"acc{g}a"),
                    pspool.tile([DH + 1, 512], f32, tag="acc", name=f"acc{g}b"),
                ]
                started = [False, False]
                avq = []  # (acc_idx, v_ap, p_ap)

                def drain(keep, accs=accs, started=started, avq=avq,
                          final=False):
                    while len(avq) > keep:
                        ai, vs, ps_ = avq.pop(0)
                        is_last = final and not any(q[0] == ai for q in avq)
                        nc.tensor.matmul(accs[ai][:], vs, ps_,
                                         start=(not started[ai]),
                                         stop=is_last)
                        started[ai] = True

                for jp in range(16):
                    c0, c1 = 2 * jp, 2 * jp + 1
                    j0 = slice(c0 * 128, (c0 + 1) * 128)
                    j1 = slice(c1 * 128, (c1 + 1) * 128)
                    if jp == 1 and pending_ep is not None:
                        pending_ep()
                        pending_ep = None
                    # AVs from ~2 jps ago first: they fill the PE while this
                    # jp's sim tiles wait on exp completions
                    if jp % 2 == 1 or jp == 14:
                        drain(4 if jp == 14 else 8)
                    # sims: kk chunk stationaries each serve both i-tiles
                    pst = [
                        pspool.tile([128, 1024], f32, tag="sim", bufs=3,
                                    name=f"pst{g}_{jp}_{ti}")
                        for ti in (0, 1)
                    ]
                    for ti, si in ((0, sA), (1, sB)):
                        nc.tensor.matmul(pst[ti][:, 0:512], kkT[0:64, j0],
                                         qqT[0:64, si], start=True, stop=True)
                    for ti, si in ((0, sA), (1, sB)):
                        nc.tensor.matmul(pst[ti][:, 512:1024], kkT[64:128, j1],
                                         qqT[64:128, si], start=True, stop=True)
                    # exps
                    for ti in (0, 1):
                        t_glob = g * 32 + jp * 2 + ti
                        m = t_glob % 8
                        p_sb = spool.tile([128, 1024], f16, tag="p",
                                          bufs=10, name=f"p{g}_{jp}_{ti}")
                        if (t_glob % 2 == 0) or (t_glob % 16 == 7):
                            nc.scalar.activation(p_sb[:], pst[ti][:], Exp,
                                                 scale=SCALE)
                        else:
                            nc.vector.tensor_scalar(
                                p_sb[:].bitcast(i16), pst[ti][:],
                                A16, B16, mult, addop)
                        avq.append((ti, v_sb[:, c0 * 65:c0 * 65 + 65],
                                    p_sb[:, 0:512]))
                        avq.append((ti, v_sb[:, c1 * 65:c1 * 65 + 65],
                                    p_sb[:, 512:1024]))
                drain(0, final=True)

                def make_ep(g, accs):
                    def ep():
                        for ti in (0, 1):
                            osb = spool.tile([DH + 1, 512], f16, tag="osb",
                                             bufs=4, name=f"osb{g}_{ti}")
                            if ti == 0:
                                nc.scalar.copy(osb[:], accs[ti][:])
                            else:
                                nc.vector.tensor_copy(osb[:], accs[ti][:])
                            it = 2 * g + ti
                            nc.sync.dma_start(
                                oT_d[:, it * 512:(it + 1) * 512], osb[:])
                    return ep

                pending_ep = make_ep(g, accs)
            pending_ep()

    nc.compile()
    return nc


def make_in_maps(x, w_qkv):
    bf = ml_dtypes.bfloat16
    xf = np.asarray(x, np.float32).reshape(N_TOK, C_IN)
    xT = np.ascontiguousarray(xf.T).astype(bf)
    w_qkv = np.asarray(w_qkv, np.float32)
    in_maps = []
    for h in range(HEADS):
        wq = w_qkv[:, h * DH:(h + 1) * DH]
        wk = w_qkv[:, 512 + h * DH:512 + (h + 1) * DH]
        wv = w_qkv[:, 1024 + h * DH:1024 + (h + 1) * DH]
        wqkv_np = np.concatenate(
            [wq[:128], wq[128:], wk[:128], wk[128:], wv[:128], wv[128:]],
            axis=1).astype(bf)
        in_maps.append({"xT": xT, "wqkv": wqkv_np})
    return in_maps


def postprocess(results, w_out, b_out):
    """Host epilogue: normalize, project, sum heads, add bias."""
    w_out = np.asarray(w_out, np.float32)
    acc = np.zeros((N_TOK, C_IN), np.float32)
    for h in range(HEADS):
        o = np.asarray(results[h]["oT"], dtype=np.float32)
        oT = o[0:DH]                      # [64, n] unnormalized
        den = o[DH]                       # [n]
        acc += (oT / den[None, :]).T @ w_out[h * DH:(h + 1) * DH, :]
    out = acc + np.asarray(b_out, np.float32)[None, :]
    return out.astype(np.float32).reshape(1, 8, 16, 32, C_IN)


def kernel(x, w_qkv, w_out, b_out):
    from concourse.bass_utils import run_bass_kernel_spmd

    nc = _CACHE.get("nc")
    if nc is None:
        nc = build_nc()
        _CACHE["nc"] = nc
    in_maps = make_in_maps(x, w_qkv)
    res = run_bass_kernel_spmd(nc, in_maps, core_ids=list(range(N_CORES)))
    return postprocess(res.results, w_out, b_out)
